# revision 100
# baseline (speedup 1.0000x reference)
"""Distributed causal attention kernel for 8 TRN2 NeuronCores.

Sharding: core c handles batch b = c//4 and heads [8*(c%4), 8*(c%4)+8).
No collectives: each core computes the partial output projection through its
own 256-row slice of Wproj (y_head @ Wproj.T restricted to this core's head
features) and the host sums the four per-batch bf16 partials in fp32.

Pipeline per 512-token chunk i: token-major Q/K/V projections (all three into
one psum ring tile per 128-token block; RMS-norm via DVE Quake-rsqrt, rope via
sliced muls on slim [T, 32] tables), PE-transpose q/k to feature-major,
block-causal scores with column-trimmed diagonal tiles + negmask triangle,
softmax exp split across engines (ACT table exp; DVE/Pool drain + DVE bf16
Schraudolph bit-trick exp), transposed PV (pt stationary, v moving ->
out [q, 33]) with the softmax denominator as column 32, per-partition
reciprocal normalize. Chunk i+1's projections and chunk i-1's output
projection are woven into chunk i's attention as PE fillers.

PSUM discipline: slots pad to full 2 KiB banks and rotating slices of one
tile serialize against their readers (whole-tile WAR), so everything runs on
two fresh-tile rings: `ss` (3 x 2 banks: score pairs + qkv blocks) and `flex`
(2 x 1 bank: PV accumulators, transpose pairs via bf16 views, out-proj
pieces).
"""

import os
import numpy as np
import ml_dtypes

import concourse.bass as bass
import concourse.tile as tile
from concourse import bacc, mybir
from concourse.bass_utils import run_bass_kernel_spmd
from concourse.alu_op_type import AluOpType
import bass_rust as _br

B, T, D, NH, HD = 2, 2048, 1024, 32, 32
EPS = 1e-6
NCORES = 8
NEG = -30.0
SCALE = HD ** -0.5

BF16 = mybir.dt.bfloat16
F32 = mybir.dt.float32
I16 = mybir.dt.int16
I32 = mybir.dt.int32
AF = mybir.ActivationFunctionType
AX = _br.AxisListType

# bf16 Schraudolph exp: i16 = A16*s + B16, bitcast to bf16 ~= exp(SCALE*s)
SCHRA_A16 = (2 ** 7 / np.log(2.0)) * SCALE
SCHRA_B16 = 127 * 2 ** 7 - 486411.0 / 2 ** 16

# engine schedule knobs (A=ACT/scalar, D=DVE/vector, P=Pool/gpsimd).
# NOTE: GPSIMD/Pool cannot access PSUM on TRN2 hardware - only SBUF-side ops
# (square, reduce, rope halves, Schraudolph tensor_scalar) may go to "P".
EXP_PATTERNS = {0: "A", 1: "AAPAAP", 2: "AAPAAP", 3: "AADAAD"}  # per-chunk full-group round-robin
TR_PATTERN = "DD"          # q/k transpose drains (psum: A/D only)
V_DRAIN = "D"              # v psum drain (A/D only)
QR_DRAIN = "D"             # qraw psum drain (A/D only)
OT_PATTERN = "DA"          # out-proj staging drains (A/D only)
YN_ENG = "D"               # y normalize (psum: A/D only)
SQ_ENG = "P"               # rms square (sbuf)
RED_ENG = "D"              # rms reduce (free-axis: DVE only)
T2_ENG = "P"               # rope swap-half muls (sbuf)


_cache = {}


def _build(gains=None):
    nc = bacc.Bacc("TRN2", target_bir_lowering=False, debug=False, num_devices=NCORES)

    xT = nc.dram_tensor("xT", [D, T], BF16, kind="ExternalInput")
    wqT = nc.dram_tensor("wqT", [D, 256], BF16, kind="ExternalInput")
    wkT = nc.dram_tensor("wkT", [D, 256], BF16, kind="ExternalInput")
    wvT = nc.dram_tensor("wvT", [D, 256], BF16, kind="ExternalInput")
    wpT = nc.dram_tensor("wpT", [256, D], BF16, kind="ExternalInput")
    ctab_d = nc.dram_tensor("ctab", [T, 32], BF16, kind="ExternalInput")
    stab_d = nc.dram_tensor("stab", [T, 32], BF16, kind="ExternalInput")
    ident = nc.dram_tensor("ident", [128, 128], BF16, kind="ExternalInput")
    negmask = nc.dram_tensor("negmask", [128, 128], BF16, kind="ExternalInput")
    gains_d = None
    if gains is not None:
        gains_d = nc.dram_tensor("gains", [128, 16], F32, kind="ExternalInput")
    out = nc.dram_tensor("out", [8, 128, T], BF16, kind="ExternalOutput")

    with tile.TileContext(nc) as tc:
        with (
            tc.tile_pool(name="const", bufs=1) as cpool,
            tc.tile_pool(name="persist", bufs=1) as ppool,
            tc.tile_pool(name="small", bufs=10) as spool_sb,
            tc.tile_pool(name="work", bufs=8) as wpool,
            tc.tile_pool(name="ptp", bufs=24) as ptpool,
            tc.tile_pool(name="ps", bufs=1, space="PSUM") as pspool,
        ):
            ENG = {"A": nc.scalar, "D": nc.vector, "P": nc.gpsimd}

            def copy_on(eng, out_, in_):
                if eng == "A":
                    nc.scalar.copy(out=out_, in_=in_)
                else:
                    ENG[eng].tensor_copy(out=out_, in_=in_)

            # ---- constants / inputs ----
            ident_sb = cpool.tile([128, 128], BF16, tag="ident")
            nc.gpsimd.dma_start(out=ident_sb[:], in_=ident[:, :])
            nm_sb = cpool.tile([128, 128], BF16, tag="negmask")
            nc.gpsimd.dma_start(out=nm_sb[:], in_=negmask[:, :])
            gains_sb = None
            if gains_d is not None:
                gains_sb = cpool.tile([128, 16], F32, tag="gains")
                nc.gpsimd.dma_start(out=gains_sb[:], in_=gains_d[:, :])

            w_sb = {}
            for name in ("wq", "wk", "wv"):
                w_sb[name] = cpool.tile([128, 8, 256], BF16, tag=name, name=f"w_{name}")
            wp_sb = cpool.tile([128, 2, D], BF16, tag="wp")
            xT_sb = cpool.tile([128, 8, T], BF16, tag="xT")
            tab = {}
            for name in ("c", "s"):
                tab[name] = cpool.tile([128, 16, 32], BF16, tag=name, name=f"tab_{name}")

            xr = xT.ap().rearrange("(kc p) t -> p kc t", p=128)
            # split the first loads so the first q matmuls can start sooner
            wqr = wqT.ap().rearrange("(kc p) f -> p kc f", p=128)
            nc.sync.dma_start(out=xT_sb[:, 0:4, 0:128], in_=xr[:, 0:4, 0:128])
            nc.sync.dma_start(out=w_sb["wq"][:, 0:4, :], in_=wqr[:, 0:4, :])
            nc.sync.dma_start(out=xT_sb[:, 4:8, 0:128], in_=xr[:, 4:8, 0:128])
            nc.sync.dma_start(out=w_sb["wq"][:, 4:8, :], in_=wqr[:, 4:8, :])
            nc.sync.dma_start(out=w_sb["wk"][:], in_=wkT.ap().rearrange("(kc p) f -> p kc f", p=128))
            for name, dram_t in (("c", ctab_d), ("s", stab_d)):
                nc.sync.dma_start(
                    out=tab[name][:], in_=dram_t.ap().rearrange("(tt p) f -> p tt f", p=128)
                )
            for tt in range(1, 4):
                nc.sync.dma_start(out=xT_sb[:, :, 128 * tt : 128 * (tt + 1)], in_=xr[:, :, 128 * tt : 128 * (tt + 1)])
            nc.sync.dma_start(out=w_sb["wv"][:], in_=wvT.ap().rearrange("(kc p) f -> p kc f", p=128))
            for i in range(1, 4):
                nc.sync.dma_start(out=xT_sb[:, :, 512 * i : 512 * (i + 1)], in_=xr[:, :, 512 * i : 512 * (i + 1)])
            nc.sync.dma_start(out=wp_sb[:], in_=wpT.ap().rearrange("(kc p) f -> p kc f", p=128))

            # ---- persistent activations ----
            q_fm = ppool.tile([128, 2, T], BF16, tag="q_fm")
            k_fm = ppool.tile([128, 2, T], BF16, tag="k_fm")
            v_sb = ppool.tile([128, 16, 8, 33], BF16, tag="v_sb")
            nc.vector.memset(v_sb[:, :, :, 32:33], 1.0)
            y_sb = ppool.tile([128, 4, 4, 8, 32], BF16, tag="y_sb")  # [q128, i, qt, h, f]

            # ---- psum rings: 3x2-bank ss + 2x1-bank flex = 8 banks ----
            def ss_tile(nm):
                return pspool.tile([128, 1024], F32, tag="ss", name=nm, bufs=3)

            def flex_tile(nm):
                return pspool.tile([128, 512], F32, tag="flex", name=nm, bufs=2)

            # ---------------- phase A: qkv projections for one 128-token block tt ----------------
            _qraw = {}

            def emit_qkv(tt):
                """Q/K/V projection matmuls into one psum ring tile
                (q at [0:256], k at [256:512], v at [512:768]) + drains."""
                ps = ss_tile(f"qkv_{tt}")
                for col, wname in ((0, "wq"), (256, "wk"), (512, "wv")):
                    for kc in range(8):
                        nc.tensor.matmul(
                            ps[:, col : col + 256],
                            xT_sb[:, kc, 128 * tt : 128 * (tt + 1)],
                            w_sb[wname][:, kc, :],
                            start=(kc == 0),
                            stop=(kc == 7),
                        )
                    if col == 256:
                        # drain q|k while PE continues with the v matmuls
                        qraw = wpool.tile([128, 2, 256], BF16, tag="qraw", name=f"qraw_{tt}")
                        copy_on("A", qraw[:], ps[:, 0:512].rearrange("p (pk f) -> p pk f", pk=2))
                        _qraw[tt] = qraw
                copy_on("A", v_sb[:, tt, :, 0:32], ps[:, 512:768].rearrange("p (h f) -> p h f", h=8))

            _msb = {}
            _pq = {}

            def emit_rms_stats(tt):
                """one wide square + one segmented reduce over [128, 2, 8, 32]
                into the 4-tt batched ms tile."""
                pq = _qraw.pop(tt)
                _pq[tt] = pq
                sq = wpool.tile([128, 2, 256], BF16, tag="sq", name=f"sq_{tt}")
                ENG[SQ_ENG].tensor_tensor(out=sq[:], in0=pq[:], in1=pq[:], op=AluOpType.mult)
                cs = tt // 2
                if cs not in _msb:
                    _msb[cs] = spool_sb.tile([128, 2, 16], F32, tag="msb", name=f"msb_{cs}", bufs=2)
                ENG[RED_ENG].tensor_reduce(
                    out=_msb[cs][:, tt % 2, :].rearrange("p (pk h) -> p pk h", pk=2).unsqueeze(-1),
                    in_=sq[:].rearrange("p pk (h f) -> p pk h f", h=8),
                    axis=AX.X,
                    op=AluOpType.add,
                )

            def emit_quake4(cs):
                """Quake rsqrt over one 2-tt ms batch in wide [128, 32] ops."""
                msb = _msb.pop(cs)
                vv = spool_sb.tile([128, 32], F32, tag="vv", name=f"vv_{cs}")
                nc.vector.tensor_scalar(out=vv[:], in0=msb[:].rearrange("p a b -> p (a b)"),
                                        scalar1=1.0 / HD, scalar2=EPS,
                                        op0=AluOpType.mult, op1=AluOpType.add)
                y0 = spool_sb.tile([128, 32], I32, tag="y0", name=f"y0_{cs}")
                nc.vector.tensor_scalar(out=y0[:], in0=vv[:].bitcast(I32), scalar1=1,
                                        scalar2=0, op0=AluOpType.logical_shift_right,
                                        op1=AluOpType.logical_shift_right)
                nc.vector.tensor_scalar(out=y0[:], in0=y0[:], scalar1=-1,
                                        scalar2=0x5F3759DF, op0=AluOpType.mult,
                                        op1=AluOpType.add)
                t4 = spool_sb.tile([128, 32], F32, tag="t4", name=f"t4_{cs}")
                nc.vector.tensor_tensor(out=t4[:], in0=y0[:].bitcast(F32), in1=y0[:].bitcast(F32), op=AluOpType.mult)
                nc.vector.tensor_tensor(out=t4[:], in0=t4[:], in1=vv[:], op=AluOpType.mult)
                nc.vector.tensor_scalar(out=t4[:], in0=t4[:], scalar1=-0.5, scalar2=1.5,
                                        op0=AluOpType.mult, op1=AluOpType.add)
                m = spool_sb.tile([128, 32], BF16, tag="m", name=f"m_{cs}", bufs=3)
                nc.vector.tensor_tensor(out=m[:], in0=y0[:].bitcast(F32), in1=t4[:], op=AluOpType.mult)
                if gains_sb is not None:
                    nc.vector.tensor_tensor(
                        out=m[:].rearrange("p (a b) -> p a b", a=2),
                        in0=m[:].rearrange("p (a b) -> p a b", a=2),
                        in1=gains_sb[:].unsqueeze(1).broadcast_to([128, 2, 16]),
                        op=AluOpType.mult,
                    )
                mv = m[:].rearrange("p (a b) -> p a b", a=2)
                for t4i in range(2):
                    _ms_m[2 * cs + t4i] = mv[:, t4i, :]

            _ms_m = {}
            _rope_out = {}

            def emit_rms_apply(tt, g2s=(0, 1)):
                """qn = qraw * m, then rope, on [128, 2, 4, 32] views."""
                pq = _pq[tt]
                m = _ms_m[tt]
                ct = tab["c"][:, tt, :].unsqueeze(1).broadcast_to([128, 2, 32]).unsqueeze(2).broadcast_to([128, 2, 4, 32])
                st = tab["s"][:, tt, :].unsqueeze(1).broadcast_to([128, 2, 32]).unsqueeze(2).broadcast_to([128, 2, 4, 32])
                for g2 in g2s:
                    fs = slice(128 * g2, 128 * (g2 + 1))
                    pq4 = pq[:, :, fs].rearrange("p pk (h f) -> p pk h f", h=4)
                    qn = wpool.tile([128, 2, 4, 32], BF16, tag="qn", name=f"qn_{tt}g{g2}")
                    nc.vector.tensor_tensor(
                        out=qn[:],
                        in0=pq4,
                        in1=m.rearrange("p (pk h) -> p pk h", pk=2)[:, :, 4 * g2 : 4 * g2 + 4].unsqueeze(-1).broadcast_to([128, 2, 4, 32]),
                        op=AluOpType.mult,
                    )
                    t1 = wpool.tile([128, 2, 4, 32], BF16, tag="t1", name=f"t1_{tt}g{g2}")
                    nc.vector.tensor_tensor(out=t1[:], in0=qn[:], in1=ct, op=AluOpType.mult)
                    t2 = wpool.tile([128, 2, 4, 32], BF16, tag="t2", name=f"t2_{tt}g{g2}")
                    qn4 = qn[:].rearrange("p pk h (a f) -> p pk h a f", a=2)
                    st4 = st.rearrange("p pk h (a f) -> p pk h a f", a=2)
                    t24 = t2[:].rearrange("p pk h (a f) -> p pk h a f", a=2)
                    for a in range(2):
                        ENG[T2_ENG].tensor_tensor(out=t24[:, :, :, a, :], in0=qn4[:, :, :, 1 - a, :],
                                                  in1=st4[:, :, :, a, :], op=AluOpType.mult)
                    rp = wpool.tile([128, 2, 4, 32], BF16, tag="rp", name=f"rp_{tt}g{g2}")
                    nc.vector.tensor_tensor(out=rp[:], in0=t1[:], in1=t2[:], op=AluOpType.add)
                    _rope_out[(tt, g2)] = rp

            _tr_ctr = [0]

            def emit_transpose_g2(tt, pname, g2):
                """Single-half transpose via a flex tile (chunk-0 staging)."""
                dst = q_fm if pname == "q" else k_fm
                pk = 0 if pname == "q" else 1
                rp = _rope_out[(tt, g2)]
                fx = flex_tile(f"tr_{pname}{tt}g{g2}")
                fxb = fx[:].bitcast(BF16)
                nc.tensor.transpose(fxb[:, 0:128], rp[:, pk, :, :].rearrange("p h f -> p (h f)"), ident_sb[:])
                eng = TR_PATTERN[_tr_ctr[0] % len(TR_PATTERN)]
                _tr_ctr[0] += 1
                copy_on(eng, dst[:, g2, 128 * tt : 128 * (tt + 1)], fxb[:, 0:128])

            def emit_transpose(tt, pname):
                """Both g2 halves into one flex tile + one wide drain."""
                dst = q_fm if pname == "q" else k_fm
                pk = 0 if pname == "q" else 1
                fx = flex_tile(f"tr_{pname}{tt}")
                fxb = fx[:].bitcast(BF16)
                for g2 in range(2):
                    rp = _rope_out[(tt, g2)]
                    nc.tensor.transpose(fxb[:, 128 * g2 : 128 * (g2 + 1)],
                                        rp[:, pk, :, :].rearrange("p h f -> p (h f)"), ident_sb[:])
                eng = TR_PATTERN[_tr_ctr[0] % len(TR_PATTERN)]
                _tr_ctr[0] += 1
                copy_on(eng, dst[:, :, 128 * tt : 128 * (tt + 1)],
                        fxb[:, 0:256].rearrange("p (g q) -> p g q", g=2))

            _exp_ctr = [0]

            # ---------------- filler queue: PE work woven between score groups ----------------
            fillers = []

            def pop_filler(npe=1):
                got = 0
                while fillers:
                    fn, has_pe = fillers.pop(0)
                    fn()
                    if has_pe:
                        got += 1
                        if got >= npe:
                            break

            def flush_fillers():
                while fillers:
                    fillers.pop(0)[0]()

            # ---------------- phase B: attention chunk i ----------------
            _grp_ctr = [0]

            def emit_scores(i, h):
                """Score matmuls + negmask + exp for (chunk i, head h).
                Returns list of (pt_tile, grp) for PV."""
                g2, hl = h // 4, h % 4
                njs = 4 * i + 4
                iq0 = 512 * i
                groups = [[g0, g0 + 1] for g0 in range(0, njs, 2)]
                pts = []
                for grp in groups:
                    _grp_ctr[0] += 1
                    if i == 0:
                        pop_filler(2)
                    elif i < 2 or _grp_ctr[0] % 2 == 0:
                        pop_filler(1)
                    ss = ss_tile(f"ss_{i}_{h}_{grp[0]}")
                    los = []
                    for jj, j in enumerate(grp):
                        d = j - 4 * i
                        diag = d >= 0
                        lo = 128 * d if diag else 0
                        los.append(lo)
                        nc.tensor.matmul(
                            ss[:, 512 * jj + lo : 512 * (jj + 1)],
                            k_fm[32 * hl : 32 * (hl + 1), g2, 128 * j : 128 * (j + 1)],
                            q_fm[32 * hl : 32 * (hl + 1), g2, iq0 + lo : iq0 + 512],
                            start=True,
                            stop=not diag,
                            tile_position=(32 * hl, 0),
                        )
                        if diag:
                            nc.tensor.matmul(
                                ss[:, 512 * jj + lo : 512 * jj + lo + 128],
                                ident_sb[:, :],
                                nm_sb[:, :],
                                start=False,
                                stop=True,
                            )
                    pt = ptpool.tile([128, 1024], BF16, tag="pt")
                    lo0, lo1 = los
                    pat = EXP_PATTERNS[i]
                    eng = pat[_exp_ctr[0] % len(pat)]
                    _exp_ctr[0] += 1
                    if eng != "A":
                        # drain psum f32 -> sbuf bf16 on DVE, then single-op
                        # bf16 Schraudolph exp on DVE ("D") or Pool ("P")
                        sbf = wpool.tile([128, 1024], BF16, tag=f"sbf{eng}", bufs=6 if eng == "P" else 3)
                        nc.vector.tensor_copy(out=sbf[:], in_=ss[:])
                        ENG[eng].tensor_scalar(out=pt[:].bitcast(I16), in0=sbf[:], scalar1=SCHRA_A16,
                                               scalar2=SCHRA_B16, op0=AluOpType.mult, op1=AluOpType.add)
                    else:
                        if lo1 == 0:
                            ranges = [(lo0, 1024)]
                        else:
                            ranges = [(lo0, 512), (512 + lo1, 1024)]
                        for r0, r1 in ranges:
                            nc.scalar.activation(out=pt[:, r0:r1], in_=ss[:, r0:r1], func=AF.Exp, scale=SCALE)
                    pts.append((pt, grp))
                return pts

            def emit_pv(i, h, pts):
                """Transposed PV + normalize for (chunk i, head h)."""
                fx = flex_tile(f"pv_{i}_{h}")
                ps = fx[:, 0:160].rearrange("p (qt f) -> p qt f", qt=4)
                for qt in range(4):
                    jmax = 4 * i + qt
                    for j in range(jmax + 1):
                        g = j // 2
                        pt, grp = pts[g]
                        col = 512 * (j - grp[0]) + 128 * qt
                        nc.tensor.matmul(
                            ps[:, qt, 0:33],
                            pt[:, col : col + 128],
                            v_sb[:, j, h, :],
                            start=(j == 0),
                            stop=(j == jmax),
                        )
                rd = spool_sb.tile([128, 4], F32, tag="rd", name=f"rd_{i}_{h}")
                nc.vector.reciprocal(out=rd[:], in_=ps[:, :, 32:33].squeeze(-1))
                ENG[YN_ENG].tensor_tensor(
                    out=y_sb[:, i, :, h, :],
                    in0=ps[:, :, 0:32],
                    in1=rd[:].unsqueeze(-1).broadcast_to([128, 4, 32]),
                    op=AluOpType.mult,
                )

            # ---------------- phase C: output projection for chunk i ----------------
            _yfm = {}
            _ot_ctr = [0]

            def emit_ytr(i, fh):
                """PE-transpose y chunk i, feature-half fh: 2 flex tiles of 2
                transposes each + 2 wide drains into the chunk's fm tile."""
                if (i,) not in _yfm:
                    _yfm[(i,)] = wpool.tile([128, 2, 512], BF16, tag="yfm", name=f"yfm_{i}", bufs=2)
                yfm = _yfm[(i,)]
                for half in range(2):
                    fx = flex_tile(f"ytr_{i}f{fh}h{half}")
                    fxb = fx[:].bitcast(BF16)
                    for qq in range(2):
                        qt = 2 * half + qq
                        nc.tensor.transpose(
                            fxb[:, 128 * qq : 128 * (qq + 1)],
                            y_sb[:, i, qt, 4 * fh : 4 * fh + 4, :].rearrange("p h f -> p (h f)"),
                            ident_sb[:],
                        )
                    nc.vector.tensor_copy(out=yfm[:, fh, 256 * half : 256 * (half + 1)], in_=fxb[:, 0:256])

            def emit_proj_mt(i, mt):
                """Partial out-proj for 128 d_out rows x 512 tokens: one flex
                psum piece, bf16 staging copy, one DMA."""
                yfm = _yfm[(i,)]
                fx = flex_tile(f"po_{i}_{mt}")
                for kc in range(2):
                    nc.tensor.matmul(
                        fx[:],
                        wp_sb[:, kc, 128 * mt : 128 * (mt + 1)],
                        yfm[:, kc, :],
                        start=(kc == 0),
                        stop=(kc == 1),
                    )
                ot = wpool.tile([128, 512], BF16, tag="ot", name=f"ot_{i}_{mt}", bufs=6)
                eng = OT_PATTERN[_ot_ctr[0] % len(OT_PATTERN)]
                _ot_ctr[0] += 1
                copy_on(eng, ot[:], fx[:])
                nc.sync.dma_start(
                    out=out[mt, :, 512 * i : 512 * (i + 1)],
                    in_=ot[:],
                )

            def c_phase_items(i):
                items = []
                for fh in range(2):
                    items.append((lambda ii=i, f=fh: emit_ytr(ii, f), True))
                for mt in range(8):
                    items.append((lambda ii=i, m=mt: emit_proj_mt(ii, m), True))
                return items

            # ================= emission =================
            # phase A for chunk 0, standalone: staged so DVE latency is hidden
            emit_qkv(0)
            emit_qkv(1)
            emit_rms_stats(0)
            emit_qkv(2)
            emit_rms_stats(1)
            emit_quake4(0)
            emit_qkv(3)
            emit_rms_stats(2)
            emit_rms_stats(3)
            emit_quake4(1)
            emit_rms_apply(0, (0,))
            emit_rms_apply(1, (0,))
            emit_rms_apply(2, (0,))
            emit_rms_apply(3, (0,))
            for tt in range(4):
                emit_transpose_g2(tt, "q", 0)
                emit_transpose_g2(tt, "k", 0)
            # g2=1 halves as fillers popped during B(0) heads 0-3
            for tt in range(4):
                fillers.append((lambda t=tt: emit_rms_apply(t, (1,)), False))
                fillers.append((lambda t=tt: emit_transpose_g2(t, "q", 1), True))
                fillers.append((lambda t=tt: emit_transpose_g2(t, "k", 1), True))

            def a_phase_items(tts):
                """Filler items for projecting chunk tts..tts+4."""
                t0, t1, t2, t3 = tts, tts + 1, tts + 2, tts + 3
                return [
                    (lambda: emit_qkv(t0), True),
                    (lambda: emit_qkv(t1), True),
                    (lambda: emit_rms_stats(t0), False),
                    (lambda: emit_qkv(t2), True),
                    (lambda: emit_rms_stats(t1), False),
                    (lambda: emit_quake4(tts // 2), False),
                    (lambda: emit_qkv(t3), True),
                    (lambda: emit_rms_stats(t2), False),
                    (lambda: emit_rms_stats(t3), False),
                    (lambda: emit_quake4(tts // 2 + 1), False),
                    (lambda: emit_rms_apply(t0), False),
                    (lambda: emit_transpose(t0, "q"), True),
                    (lambda: emit_transpose(t0, "k"), True),
                    (lambda: emit_rms_apply(t1), False),
                    (lambda: emit_transpose(t1, "q"), True),
                    (lambda: emit_transpose(t1, "k"), True),
                    (lambda: emit_rms_apply(t2), False),
                    (lambda: emit_transpose(t2, "q"), True),
                    (lambda: emit_transpose(t2, "k"), True),
                    (lambda: emit_rms_apply(t3), False),
                    (lambda: emit_transpose(t3, "q"), True),
                    (lambda: emit_transpose(t3, "k"), True),
                ]

            _pv_carry = [None]
            for i in range(4):
                if i < 3:
                    fillers.extend(a_phase_items(4 * (i + 1)))
                for h in range(8):
                    if h == 1 and i >= 1:
                        fillers.extend(c_phase_items(i - 1))
                    if h == 6 and i == 3:
                        # heads 0-3 of the last chunk are normalized by now;
                        # start its fh=0 transposes early
                        fillers.append((lambda: emit_ytr(3, 0), True))
                    pts = emit_scores(i, h)
                    if _pv_carry[0] is not None:
                        emit_pv(*_pv_carry[0])
                    _pv_carry[0] = (i, h, pts)
                flush_fillers()
            emit_pv(*_pv_carry[0])
            _pv_carry[0] = None
            emit_ytr(3, 1)
            for mt in range(8):
                emit_proj_mt(3, mt)

    nc.compile()
    return nc


def _host_prep(x, Wq, Wk, Wv, Wproj, q_gain, cos, sin):
    bf = ml_dtypes.bfloat16

    # slim rope tables [T, 32]: col f -> cos[t, f%16]; sin sign: -sin for
    # f<16, +sin for f>=16. Per-head gain (if any != 1) applied to the q
    # rms-norm multiplier on device.
    cos16 = np.asarray(cos, np.float32)  # [T, 16]
    sin16 = np.asarray(sin, np.float32)
    c32 = np.concatenate([cos16, cos16], 1)  # [T, 32]
    s32 = np.concatenate([-sin16, sin16], 1)

    gall = np.asarray(q_gain, np.float32)
    ones = bool(np.allclose(gall, 1.0))

    ident = np.eye(128, dtype=np.float32)
    negmask = np.zeros((128, 128), np.float32)
    neg_raw = NEG / SCALE
    for mm in range(128):
        negmask[mm, 0:mm] = neg_raw

    consts = dict(
        ident=ident.astype(bf),
        negmask=negmask.astype(bf),
        ctab=c32.astype(bf),
        stab=s32.astype(bf),
    )

    in_maps = []
    for c in range(NCORES):
        b, g = c // 4, c % 4
        rows = slice(256 * g, 256 * (g + 1))
        m = dict(consts)
        m["xT"] = np.ascontiguousarray(np.asarray(x, np.float32)[b].T).astype(bf)
        m["wqT"] = np.ascontiguousarray(np.asarray(Wq, np.float32)[rows].T).astype(bf)
        m["wkT"] = np.ascontiguousarray(np.asarray(Wk, np.float32)[rows].T).astype(bf)
        m["wvT"] = np.ascontiguousarray(np.asarray(Wv, np.float32)[rows].T).astype(bf)
        m["wpT"] = np.ascontiguousarray(np.asarray(Wproj, np.float32).T[rows]).astype(bf)
        if not ones:
            gv = np.zeros((128, 16), np.float32)
            gv[:, 0:8] = gall[8 * g : 8 * (g + 1)][None, :]
            gv[:, 8:16] = 1.0
            m["gains"] = gv
        in_maps.append(m)
    return in_maps, ones


def kernel(x, Wq, Wk, Wv, Wproj, q_gain, cos, sin):
    in_maps, ones = _host_prep(x, Wq, Wk, Wv, Wproj, q_gain, cos, sin)
    key = ("nc", ones)
    if key not in _cache:
        _cache[key] = _build(gains=None if ones else True)
    nc = _cache[key]
    _cache["nc"] = nc  # test.py TimelineSim fallback reads this
    trace = bool(int(os.environ.get("KERNEL_TRACE", "0")))
    res = run_bass_kernel_spmd(nc, in_maps, core_ids=list(range(NCORES)), trace=trace)
    _cache["last_result"] = res
    full = np.zeros((B, T, D), np.float32)
    for c in range(NCORES):
        o = np.asarray(res.results[c]["out"]).astype(np.float32)  # [8, 128, T]
        full[c // 4] += o.reshape(D, T).T
    return full
